# revision 1
# baseline (speedup 1.0000x reference)
"""Multi-head causal attention (B=2, S=2048, D=1024, H=16, Dh=64) on 8 TRN2 cores.

Sharding: core = (b, g) with b = batch (2), g = head-group (4 heads each).
Each core computes QKV projections for its batch against its 4 heads' weight
columns, causal flash-style attention for those heads, and the partial output
projection against its 4 heads' Wo rows.  Host sums the 4 partials per batch
and adds the bias (the "unshard" for this row-sharded tensor-parallel split).

All matmuls run in bf16 with fp32 PSUM accumulation (validated ~3.4e-3 l2 rel
err vs the f32 reference).  Layouts are chosen so no on-chip transposes of
activations are needed except tiny per-band ones for softmax normalization:
  xT [D, S] fed pre-transposed from host -> projections produce Q^T/K^T
  scores computed transposed [k, q] so exp output feeds the AV matmul directly
  V carries an appended ones-column so context row 64 accumulates softmax sums

Emission is band-major over the 4 query bands: project QKV for band j, then
attention for band j (which causally only needs K/V bands <= j), then its
output projection -- so softmax (ACT) and matmul (PE) work overlap from the
start instead of serializing into phases.
"""

import numpy as np
import ml_dtypes

B = 2
S = 2048
D = 1024
HPC = 4  # heads per core
DH = 64
QB = 512  # q band width
NB = S // QB  # 4 bands
KT = 128  # k tile
N_CORES = 8

_CACHE = {}


def _build_bass():
    import concourse.bacc as bacc
    import concourse.tile as tile
    from concourse import mybir

    f32 = mybir.dt.float32
    bf16 = mybir.dt.bfloat16

    nc = bacc.Bacc("TRN2", target_bir_lowering=False)

    xT_d = nc.dram_tensor("xT", [D, S], bf16, kind="ExternalInput")
    wqkv_d = nc.dram_tensor("wqkv", [D, 768], bf16, kind="ExternalInput")
    wo_d = nc.dram_tensor("wo", [256, D], bf16, kind="ExternalInput")
    masks_d = nc.dram_tensor("masks", [128, 4, QB], bf16, kind="ExternalInput")
    ident_d = nc.dram_tensor("ident", [128, 128], bf16, kind="ExternalInput")
    out_d = nc.dram_tensor("out", [S, D], bf16, kind="ExternalOutput")

    KD = D // 128  # 8 contraction tiles for the projections
    ExpF = mybir.ActivationFunctionType.Exp

    with tile.TileContext(nc) as tc:
        with (
            tc.tile_pool(name="consts", bufs=1) as consts,
            tc.tile_pool(name="persist", bufs=1) as persist,
            tc.tile_pool(name="big_psum", bufs=2, space="PSUM") as big_psum,
            tc.tile_pool(name="misc_psum", bufs=2, space="PSUM") as misc_psum,
            tc.tile_pool(name="ctx_psum", bufs=2, space="PSUM") as ctx_psum,
            tc.tile_pool(name="attn_pool", bufs=8) as attn_pool,
            tc.tile_pool(name="norm_pool", bufs=4) as norm_pool,
            tc.tile_pool(name="cf_pool", bufs=8) as cf_pool,
            tc.tile_pool(name="out_pool", bufs=4) as out_pool,
        ):
            # ---- load constants: per-k weight+x slices first so the first
            #      projection group starts ASAP; masks/ident follow (they are
            #      only needed once band-0 attention begins) ----
            ones_r = consts.tile([1, 64], bf16, tag="ones_r", name="ones_r")
            nc.vector.memset(ones_r, 1.0)

            xT, wqkv = [], []
            for k in range(KD):
                t = consts.tile([128, 768], bf16, tag=f"wqkv{k}", name=f"wqkv{k}")
                nc.sync.dma_start(out=t, in_=wqkv_d[k * 128 : (k + 1) * 128, :])
                wqkv.append(t)
                t = consts.tile([128, S], bf16, tag=f"xT{k}", name=f"xT{k}")
                nc.sync.dma_start(
                    out=t[:, 0:QB], in_=xT_d[k * 128 : (k + 1) * 128, 0:QB]
                )
                xT.append(t)
            ident = consts.tile([128, 128], bf16, tag="ident", name="ident")
            nc.sync.dma_start(out=ident, in_=ident_d[:, :])
            mask_sb = consts.tile([128, 4, QB], bf16, tag="masks", name="masks")
            nc.sync.dma_start(out=mask_sb, in_=masks_d[:, :, :])
            # later bands of x^T stream in behind the band-0 slices
            for j in range(1, NB):
                for k in range(KD):
                    nc.sync.dma_start(
                        out=xT[k][:, j * QB : (j + 1) * QB],
                        in_=xT_d[k * 128 : (k + 1) * 128, j * QB : (j + 1) * QB],
                    )

            wo = []
            for p in range(2):
                t = consts.tile([128, D], bf16, tag=f"wo{p}", name=f"wo{p}")
                nc.sync.dma_start(out=t, in_=wo_d[p * 128 : (p + 1) * 128, :])
                wo.append(t)

            # ---- persistent activations ----
            qT = [
                persist.tile([128, S], bf16, tag=f"qT{p}", name=f"qT{p}")
                for p in range(2)
            ]
            kTt = [
                persist.tile([128, S], bf16, tag=f"kT{p}", name=f"kT{p}")
                for p in range(2)
            ]
            vT = [
                persist.tile([128, S], bf16, tag=f"vT{p}", name=f"vT{p}")
                for p in range(2)
            ]
            v_sb = [
                persist.tile([128, S // KT, 65], bf16, tag=f"v{h}", name=f"v{h}")
                for h in range(HPC)
            ]
            ctxo = [
                persist.tile([128, S], bf16, tag=f"ctxo{p}", name=f"ctxo{p}")
                for p in range(2)
            ]
            for h in range(HPC):
                nc.vector.memset(v_sb[h][:, :, 64:65], 1.0)

            def emit_proj(j):
                """QKV projections + V transposes for band j.

                Band 0 runs before any attention, so its Q/K projections
                borrow the (idle) score-PSUM pool: all six accumulation
                groups stay open at once and consume each x^T DMA tile the
                moment it lands instead of serializing through the two misc
                slots."""
                q0 = j * QB
                for t, dest, eng in ((0, qT, "vec"), (1, kTt, "vec"), (2, vT, "vec")):
                    big0 = j == 0 and t < 2
                    if big0:
                        bps_ = big_psum.tile([128, 1024], f32, tag="big", name="projbig")
                    for p in range(2):
                        if big0:
                            ps = bps_[:, p * QB : (p + 1) * QB]
                        else:
                            ps = misc_psum.tile([128, QB], f32, tag="misc", name="projps")
                        for k in range(KD):
                            nc.tensor.matmul(
                                ps,
                                lhsT=wqkv[k][:, 256 * t + 128 * p : 256 * t + 128 * (p + 1)],
                                rhs=xT[k][:, q0 : q0 + QB],
                                start=(k == 0),
                                stop=(k == KD - 1),
                            )
                        if eng == "act":
                            nc.scalar.copy(out=dest[p][:, q0 : q0 + QB], in_=ps)
                        else:
                            nc.vector.tensor_copy(
                                out=dest[p][:, q0 : q0 + QB], in_=ps
                            )
                for h in range(HPC):
                    p, c = divmod(h, 2)
                    vtp = misc_psum.tile([128, 4, 64], bf16, tag="misc", name="vtp")
                    for t4 in range(4):
                        t = 4 * j + t4
                        nc.tensor.transpose(
                            out=vtp[:, t4, :],
                            in_=vT[p][64 * c : 64 * c + 64, t * KT : (t + 1) * KT],
                            identity=ident[
                                64 * c : 64 * c + 64, 64 * c : 64 * c + 64
                            ],
                        )
                    nc.vector.tensor_copy(
                        out=v_sb[h][:, 4 * j : 4 * j + 4, 0:64], in_=vtp
                    )

            def emit_attention(j):
                """Scores+softmax+AV for band j; returns unnormalized ctx^T
                tiles (cf, in SBUF) per (pair, head)."""
                q0 = j * QB
                nk = 4 * (j + 1)
                cfs = {}
                for p in range(2):
                    cps = [
                        ctx_psum.tile([65, QB], f32, tag="ctx", name="ctx")
                        for _ in range(2)
                    ]
                    for i2 in range(nk // 2):
                        sps = [
                            big_psum.tile([128, 1024], f32, tag="big", name="sps")
                            for _ in range(2)
                        ]
                        at = [
                            attn_pool.tile([128, 1024], bf16, tag="attn", name="at")
                            for _ in range(2)
                        ]
                        # the two heads' score matmuls use disjoint PE row
                        # groups (partitions 0-63 vs 64-127), so interleaving
                        # them lets the PE run both concurrently
                        for half in range(2):
                            for c in range(2):
                                i = 2 * i2 + half
                                o = i - 4 * j
                                z = 128 * o if o > 0 else 0
                                nc.tensor.matmul(
                                    sps[c][:, half * QB + z : (half + 1) * QB],
                                    lhsT=kTt[p][
                                        64 * c : 64 * c + 64,
                                        i * KT : (i + 1) * KT,
                                    ],
                                    rhs=qT[p][
                                        64 * c : 64 * c + 64, q0 + z : q0 + QB
                                    ],
                                    start=True,
                                    stop=True,
                                )
                        # diagonal k-tile groups: columns < 128*o are fully
                        # causally masked -- skip exp there (zero instead) and
                        # mask-multiply only the 128-wide partial strip.
                        # off-diagonal groups keep one wide exp (less ACT
                        # per-op overhead).
                        diag = 2 * i2 - 4 * j >= 0
                        for c in range(2):
                            if not diag:
                                nc.scalar.activation(
                                    out=at[c], in_=sps[c], func=ExpF
                                )
                            else:
                                for half in range(2):
                                    o = 2 * i2 + half - 4 * j
                                    z = 128 * o if o > 0 else 0
                                    sl = slice(half * QB + z, (half + 1) * QB)
                                    nc.scalar.activation(
                                        out=at[c][:, sl],
                                        in_=sps[c][:, sl],
                                        func=ExpF,
                                    )
                                for half in range(2):
                                    o = 2 * i2 + half - 4 * j
                                    z = 128 * o
                                    w = min(z + 128, QB)
                                    nc.vector.tensor_mul(
                                        at[c][:, half * QB + z : half * QB + w],
                                        at[c][:, half * QB + z : half * QB + w],
                                        mask_sb[:, o, z:w],
                                    )
                            h = 2 * p + c
                            for half in range(2):
                                i = 2 * i2 + half
                                o = i - 4 * j
                                z = 128 * o if o > 0 else 0
                                nc.tensor.matmul(
                                    cps[c][:, z:QB],
                                    lhsT=v_sb[h][:, i, :],
                                    rhs=at[c][:, half * QB + z : (half + 1) * QB],
                                    start=(i == 0),
                                    stop=(i == nk - 1),
                                )
                    for c in range(2):
                        cf = cf_pool.tile([65, QB], bf16, tag="cf", name="cf")
                        nc.vector.tensor_copy(out=cf[0:64, :], in_=cps[c][0:64, :])
                        rr = norm_pool.tile([1, QB], bf16, tag="rr", name="rr")
                        with nc.allow_low_precision(
                            reason="reciprocal feeds a bf16 multiply"
                        ):
                            nc.vector.reciprocal(out=rr, in_=cps[c][64:65, :])
                        cfs[(p, c)] = (cf, rr)
                return cfs

            def emit_norm(j, cfs):
                """Divide ctx^T rows by the softmax sums (row 64 of cf):
                broadcast 1/sums across partitions with a rank-1 matmul
                (ones[64] x recip_row), then a single elementwise multiply
                writes the normalized bf16 ctx^T operand."""
                q0 = j * QB
                for p in range(2):
                    for c in range(2):
                        cf, rr = cfs[(p, c)]
                        rb = misc_psum.tile([64, QB], f32, tag="misc", name="rb")
                        nc.tensor.matmul(
                            rb, lhsT=ones_r, rhs=rr, start=True, stop=True
                        )
                        nc.vector.tensor_mul(
                            ctxo[p][64 * c : 64 * c + 64, q0 : q0 + QB],
                            cf[0:64, :],
                            rb,
                        )

            def emit_outproj(j):
                # last band: DMA each half as soon as its copy lands (the
                # queues are empty by then and the drain is the critical path)
                last = j == NB - 1
                for m in range(4 * j, 4 * j + 4):
                    osb = out_pool.tile([128, 1024], bf16, tag="osb", name="osb")
                    for n in range(2):
                        ops = misc_psum.tile([128, QB], f32, tag="misc", name="ops")
                        for p in range(2):
                            nc.tensor.matmul(
                                ops,
                                lhsT=ctxo[p][:, m * KT : (m + 1) * KT],
                                rhs=wo[p][:, n * QB : (n + 1) * QB],
                                start=(p == 0),
                                stop=(p == 1),
                            )
                        nc.vector.tensor_copy(
                            out=osb[:, n * QB : (n + 1) * QB], in_=ops
                        )
                        if last:
                            nc.sync.dma_start(
                                out=out_d[
                                    m * KT : (m + 1) * KT, n * QB : (n + 1) * QB
                                ],
                                in_=osb[:, n * QB : (n + 1) * QB],
                            )
                    if not last:
                        nc.sync.dma_start(
                            out=out_d[m * KT : (m + 1) * KT, :], in_=osb
                        )

            # band-major pipeline: next band's projections are emitted before
            # this band's normalization tail so the PE always has matmul work
            # while softmax (ACT) catches up
            emit_proj(0)
            for j in range(NB):
                cfs = emit_attention(j)
                if j + 1 < NB:
                    emit_proj(j + 1)
                emit_norm(j, cfs)
                emit_outproj(j)

    nc.compile()
    return nc


def _get_bass():
    if "nc" not in _CACHE:
        _CACHE["nc"] = _build_bass()
    return _CACHE["nc"]


def _make_in_maps(x, Wq, Wk, Wv, Wo):
    bf = ml_dtypes.bfloat16
    if "masks" not in _CACHE:
        # causal staircase masks: keep iff q >= k + 128*o  (within a band, a
        # k-tile at offset o*128 above the band start)
        kp = np.arange(128)[:, None]
        qf = np.arange(QB)[None, :]
        _CACHE["masks"] = np.ascontiguousarray(
            np.stack(
                [(qf >= kp + 128 * o).astype(np.float32) for o in range(4)]
            ).transpose(1, 0, 2)
        ).astype(bf)
        _CACHE["ident"] = np.eye(128, dtype=np.float32).astype(bf)
    masks, ident = _CACHE["masks"], _CACHE["ident"]

    xTs = [np.ascontiguousarray(x[b].T).astype(bf) for b in range(B)]
    in_maps = []
    for core in range(N_CORES):
        b, g = divmod(core, 4)
        hs = slice(g * 256, (g + 1) * 256)
        if core < 4:
            shards = {
                # fold 1/sqrt(Dh) into the Q weights
                "wqkv": np.concatenate(
                    [Wq[:, hs] * 0.125, Wk[:, hs], Wv[:, hs]], axis=1
                ).astype(bf),
                "wo": np.ascontiguousarray(Wo[hs, :]).astype(bf),
            }
        else:
            shards = {k: in_maps[core - 4][k] for k in ("wqkv", "wo")}
        in_maps.append(
            {"xT": xTs[b], "masks": masks, "ident": ident, **shards}
        )
    return in_maps


def _run(x, Wq, Wk, Wv, Wo, bo, trace=False):
    from concourse.bass_utils import run_bass_kernel_spmd

    nc = _get_bass()
    in_maps = _make_in_maps(x, Wq, Wk, Wv, Wo)
    res = run_bass_kernel_spmd(
        nc, in_maps, core_ids=list(range(N_CORES)), trace=trace
    )
    out = np.zeros((B, S, D), np.float32)
    for core in range(N_CORES):
        out[core // 4] += res.results[core]["out"].astype(np.float32)
    out += bo.astype(np.float32)
    return out, res


def kernel(x, Wq, Wk, Wv, Wo, bo):
    x, Wq, Wk, Wv, Wo, bo = (np.asarray(a) for a in (x, Wq, Wk, Wv, Wo, bo))
    out, _ = _run(x, Wq, Wk, Wv, Wo, bo, trace=False)
    return out


def kernel_traced(x, Wq, Wk, Wv, Wo, bo):
    """Same as kernel() but returns (out, BassKernelResults) with profiling."""
    x, Wq, Wk, Wv, Wo, bo = (np.asarray(a) for a in (x, Wq, Wk, Wv, Wo, bo))
    return _run(x, Wq, Wk, Wv, Wo, bo, trace=True)



# revision 4
# speedup vs baseline: 1.0617x; 1.0617x over previous
"""Multi-head causal attention (B=2, S=2048, D=1024, H=16, Dh=64) on 8 TRN2 cores.

Sharding: core = (b, g) with b = batch (2), g = head-group (4 heads each).
Each core computes QKV projections for its batch against its 4 heads' weight
columns, causal attention for those heads, and the partial output projection
against its 4 heads' Wo rows.  Host sums the 4 partials per batch and adds
the bias.

Precision: bf16 matmuls with fp32 PSUM accumulation everywhere EXCEPT the
score matmuls, which store Q^T/K^T in fp8 (e4m3) and run in DoubleRow perf
mode: lhsT/rhs carry a stride-0 broadcast pair so one 0.5-cycle/row DR pass
contracts dh=64 twice (the doubled scores fold into the softmax exp scale
1/16).  fp8 elsewhere fails the 2e-2 gate: per-element quantization noise
(~2.7% for e4m3) passes through dot products against random data undamped,
and the independent contributions stack to ~5.5e-2.

Layouts avoid all on-chip transposes:
  x^T [128, 8k, S] k-tile-major feeds projections directly
  V is projected in [s, dh] orientation (x^T tiles as lhsT), landing
  AV-ready with an appended ones column (row 64 accumulates softmax sums)
  scores are computed transposed [k, q] so exp output feeds AV directly

Engine split: PE does matmuls only; ACT does exp only; DVE handles
PSUM-sourced copies/reciprocals and the normalization multiply; the
otherwise-idle GPSIMD does the causal staircase mask multiplies and the
1/sums partition broadcast (replacing the baseline's rank-1 PE matmuls).
"""

import numpy as np
import ml_dtypes

B = 2
S = 2048
D = 1024
HPC = 4  # heads per core
DH = 64
QB = 512  # q band width
NB = S // QB  # 4 bands
KT = 128  # k tile
N_CORES = 8

# exp(s_psum * EXP_SCALE) = exp(s_true / sqrt(DH)); the stride-0 DR pair
# doubles s_psum.
EXP_SCALE = 1.0 / 16.0

_CACHE = {}


def _build_bass():
    import concourse.bacc as bacc
    import concourse.tile as tile
    from concourse import mybir

    f32 = mybir.dt.float32
    bf16 = mybir.dt.bfloat16
    fp8 = mybir.dt.float8e4
    DR = mybir.MatmulPerfMode.DoubleRow
    ExpF = mybir.ActivationFunctionType.Exp

    nc = bacc.Bacc("TRN2", target_bir_lowering=False)

    xT_d = nc.dram_tensor("xT", [128, 8, S], bf16, kind="ExternalInput")
    wqkv_d = nc.dram_tensor("wqkv", [128, 8, 768], bf16, kind="ExternalInput")
    wo_d = nc.dram_tensor("wo", [128, 2, D], bf16, kind="ExternalInput")
    masks_d = nc.dram_tensor("masks", [128, 4, QB], bf16, kind="ExternalInput")
    out_d = nc.dram_tensor("out", [S, D], bf16, kind="ExternalOutput")

    with tile.TileContext(nc) as tc:
        with (
            tc.tile_pool(name="consts", bufs=1) as consts,
            tc.tile_pool(name="persist", bufs=1) as persist,
            tc.tile_pool(name="score_ps", bufs=2, space="PSUM") as score_ps,
            tc.tile_pool(name="ctx_ps", bufs=2, space="PSUM") as ctx_ps,
            tc.tile_pool(name="misc_ps", bufs=2, space="PSUM") as misc_ps,
            tc.tile_pool(name="at_pool", bufs=8) as at_pool,
            tc.tile_pool(name="rr_pool", bufs=4) as rr_pool,
            tc.tile_pool(name="rb_pool", bufs=4) as rb_pool,
            tc.tile_pool(name="osb_pool", bufs=4) as osb_pool,
        ):
            # ---- constants: weights first (first proj group needs them),
            #      band-0 x^T slices, masks; later x^T bands stream behind ----
            wqkv = consts.tile([128, 8, 768], bf16, tag="wqkv", name="wqkv")
            for k in range(8):
                nc.sync.dma_start(out=wqkv[:, k, :], in_=wqkv_d[:, k, :])
            xT = consts.tile([128, 8, S], bf16, tag="xT", name="xT")
            for k in range(8):
                nc.sync.dma_start(out=xT[:, k, 0:QB], in_=xT_d[:, k, 0:QB])
            mask_sb = consts.tile([128, 4, QB], bf16, tag="masks", name="masks")
            nc.sync.dma_start(out=mask_sb, in_=masks_d[:, :, :])
            for j in range(1, NB):
                for k in range(8):
                    nc.sync.dma_start(
                        out=xT[:, k, j * QB : (j + 1) * QB],
                        in_=xT_d[:, k, j * QB : (j + 1) * QB],
                    )
            wo = consts.tile([128, 2, D], bf16, tag="wo", name="wo")
            for p in range(2):
                nc.sync.dma_start(out=wo[:, p, :], in_=wo_d[:, p, :])

            # ---- persistent activations ----
            qT = [
                persist.tile([128, S], fp8, tag=f"qT{p}", name=f"qT{p}")
                for p in range(2)
            ]
            kTt = [
                persist.tile([128, S], fp8, tag=f"kT{p}", name=f"kT{p}")
                for p in range(2)
            ]
            # v: (k-position, k-tile, head-in-pair, dh + ones column)
            vp = [
                persist.tile([128, 16, 2, 65], bf16, tag=f"vp{p}", name=f"vp{p}")
                for p in range(2)
            ]
            # ctx^T, normalized: (dh-in-pair, pair, q)
            ctxo = persist.tile([128, 2, S], bf16, tag="ctxo", name="ctxo")
            for p in range(2):
                nc.gpsimd.memset(vp[p][:, :, :, 64:65], 1.0)

            def dr2(ap, n):
                """View a [64, n] slice as a stride-0 [64, 2, n] DR pair."""
                return ap.unsqueeze(1).broadcast_to([64, 2, n])

            def emit_proj(j):
                """QKV projections for band j (bf16, fp32 PSUM).

                Q^T/K^T land as fp8 [128, QB] slabs (pair rows = 2 heads x
                64 dh) feeding the DR score matmuls.  V is projected
                directly in [s, dh] orientation (x^T tiles as lhsT), so no
                on-chip transposes are needed."""
                q0 = j * QB
                for t, dest in ((0, qT), (1, kTt)):
                    for p in range(2):
                        c0 = 256 * t + 128 * p
                        ps = misc_ps.tile([128, QB], f32, tag="misc", name="pqk")
                        for k in range(8):
                            nc.tensor.matmul(
                                ps,
                                lhsT=wqkv[:, k, c0 : c0 + 128],
                                rhs=xT[:, k, q0 : q0 + QB],
                                start=(k == 0),
                                stop=(k == 7),
                            )
                        nc.vector.tensor_copy(
                            out=dest[p][:, q0 : q0 + QB], in_=ps
                        )
                for kt4 in range(4):
                    kt = 4 * j + kt4
                    for p in range(2):
                        c0 = 512 + 128 * p
                        ps = misc_ps.tile(
                            [128, 2, 64], f32, tag="misc", name="pv"
                        )
                        for k in range(8):
                            nc.tensor.matmul(
                                ps,
                                lhsT=xT[:, k, kt * KT : (kt + 1) * KT],
                                rhs=wqkv[:, k, c0 : c0 + 128],
                                start=(k == 0),
                                stop=(k == 7),
                            )
                        nc.vector.tensor_copy(
                            out=vp[p][:, kt, :, 0:64], in_=ps
                        )

            def emit_attention(j):
                """Scores+softmax+AV for band j.

                Scores land transposed ([k, q]) in a [128, 2, QB] fp32 PSUM
                tile per (pair, k-tile-pair, head); one exp covers both
                halves.  Diagonal pairs extend the odd k-tile's q-range down
                to the even tile's start so the exp stays a single strided
                instruction; the AV matmuls read per-tile causal ranges so
                the extension region is never consumed.  GPSIMD applies the
                128-wide staircase mask strips after exp."""
                q0 = j * QB
                n_i2 = 2 * (j + 1)
                cfs = {}
                for p in range(2):
                    cps = [
                        ctx_ps.tile([65, QB], f32, tag="ctx", name="ctx")
                        for _ in range(2)
                    ]
                    for i2 in range(n_i2):
                        o_e = 2 * i2 - 4 * j
                        diag = o_e >= 0
                        z_e = 128 * o_e if diag else 0
                        z_o = z_e + 128 if diag else 0
                        for c in range(2):
                            sps = score_ps.tile(
                                [128, 2, QB], f32, tag="sps", name="sps"
                            )
                            for half in range(2):
                                i = 2 * i2 + half
                                nc.tensor.matmul(
                                    sps[:, half, z_e:QB],
                                    lhsT=dr2(
                                        kTt[p][
                                            64 * c : 64 * c + 64,
                                            i * KT : (i + 1) * KT,
                                        ],
                                        KT,
                                    ),
                                    rhs=dr2(
                                        qT[p][
                                            64 * c : 64 * c + 64,
                                            q0 + z_e : q0 + QB,
                                        ],
                                        QB - z_e,
                                    ),
                                    start=True,
                                    stop=True,
                                    perf_mode=DR,
                                )
                            at = at_pool.tile(
                                [128, 2, QB], bf16, tag="at", name="at"
                            )
                            nc.scalar.activation(
                                out=at[:, :, z_e:QB],
                                in_=sps[:, :, z_e:QB],
                                func=ExpF,
                                scale=EXP_SCALE,
                            )
                            if diag:
                                nc.gpsimd.tensor_mul(
                                    at[:, 0, z_e:z_o],
                                    at[:, 0, z_e:z_o],
                                    mask_sb[:, o_e, z_e:z_o],
                                )
                                nc.gpsimd.tensor_mul(
                                    at[:, 1, z_o : z_o + 128],
                                    at[:, 1, z_o : z_o + 128],
                                    mask_sb[:, o_e + 1, z_o : z_o + 128],
                                )
                            for half in range(2):
                                i = 2 * i2 + half
                                o = i - 4 * j
                                z = 128 * o if o > 0 else 0
                                nc.tensor.matmul(
                                    cps[c][:, z:QB],
                                    lhsT=vp[p][:, i, c, :],
                                    rhs=at[:, half, z:QB],
                                    start=(i == 0),
                                    stop=(i == 4 * (j + 1) - 1),
                                )
                    cfs[p] = cps
                return cfs

            def emit_norm(j, cfs):
                """ctx rows / softmax sums (ctx PSUM row 64): DVE reciprocal
                of the sums row, GPSIMD broadcast across partitions, one DVE
                multiply writing the normalized bf16 ctx^T operand."""
                q0 = j * QB
                for p in range(2):
                    for c in range(2):
                        cps = cfs[p][c]
                        rr = rr_pool.tile([1, QB], bf16, tag="rr", name="rr")
                        with nc.allow_low_precision(
                            reason="reciprocal feeds a bf16 multiply"
                        ):
                            nc.vector.reciprocal(out=rr, in_=cps[64:65, :])
                        rbs = rb_pool.tile([64, QB], bf16, tag="rb", name="rb")
                        nc.gpsimd.partition_broadcast(rbs, rr)
                        nc.vector.tensor_mul(
                            ctxo[64 * c : 64 * c + 64, p, q0 : q0 + QB],
                            cps[0:64, :],
                            rbs,
                        )

            def emit_outproj(j):
                last = j == NB - 1
                for m in range(4 * j, 4 * j + 4):
                    osb = osb_pool.tile([128, D], bf16, tag="osb", name="osb")
                    for n in range(2):
                        ops = misc_ps.tile([128, QB], f32, tag="misc", name="ops")
                        for p in range(2):
                            nc.tensor.matmul(
                                ops,
                                lhsT=ctxo[:, p, m * KT : (m + 1) * KT],
                                rhs=wo[:, p, n * QB : (n + 1) * QB],
                                start=(p == 0),
                                stop=(p == 1),
                            )
                        nc.vector.tensor_copy(
                            out=osb[:, n * QB : (n + 1) * QB], in_=ops
                        )
                        if last:
                            nc.sync.dma_start(
                                out=out_d[
                                    m * KT : (m + 1) * KT, n * QB : (n + 1) * QB
                                ],
                                in_=osb[:, n * QB : (n + 1) * QB],
                            )
                    if not last:
                        nc.sync.dma_start(
                            out=out_d[m * KT : (m + 1) * KT, :], in_=osb
                        )

            # band-major pipeline: next band's projections are emitted before
            # this band's normalization tail so the PE always has matmul work
            # while softmax (ACT) catches up
            emit_proj(0)
            for j in range(NB):
                cfs = emit_attention(j)
                if j + 1 < NB:
                    emit_proj(j + 1)
                emit_norm(j, cfs)
                emit_outproj(j)

    nc.compile()
    return nc


def _get_bass():
    if "nc" not in _CACHE:
        _CACHE["nc"] = _build_bass()
    return _CACHE["nc"]


def _make_in_maps(x, Wq, Wk, Wv, Wo):
    bf = ml_dtypes.bfloat16
    if "masks" not in _CACHE:
        # causal staircase masks: keep iff q >= k + 128*o  (within a band, a
        # k-tile at offset o*128 above the band start)
        kp = np.arange(128)[:, None]
        qf = np.arange(QB)[None, :]
        _CACHE["masks"] = np.ascontiguousarray(
            np.stack(
                [(qf >= kp + 128 * o).astype(np.float32) for o in range(4)]
            ).transpose(1, 0, 2)
        ).astype(bf)
    masks = _CACHE["masks"]

    # x^T in k-tile-major layout: (p, k, s) = x[b][s, 128k + p]
    xTs = [
        np.ascontiguousarray(
            x[b].T.reshape(8, 128, S).transpose(1, 0, 2)
        ).astype(bf)
        for b in range(B)
    ]
    in_maps = []
    for core in range(N_CORES):
        b, g = divmod(core, 4)
        hs = slice(g * 256, (g + 1) * 256)
        if core < 4:
            wqkv_f = np.concatenate([Wq[:, hs], Wk[:, hs], Wv[:, hs]], axis=1)
            shards = {
                "wqkv": np.ascontiguousarray(
                    wqkv_f.reshape(8, 128, 768).transpose(1, 0, 2)
                ).astype(bf),
                "wo": np.ascontiguousarray(
                    Wo[hs, :].reshape(2, 128, D).transpose(1, 0, 2)
                ).astype(bf),
            }
        else:
            shards = {k: in_maps[core - 4][k] for k in ("wqkv", "wo")}
        in_maps.append({"xT": xTs[b], "masks": masks, **shards})
    return in_maps


def _run(x, Wq, Wk, Wv, Wo, bo, trace=False):
    from concourse.bass_utils import run_bass_kernel_spmd

    nc = _get_bass()
    in_maps = _make_in_maps(x, Wq, Wk, Wv, Wo)
    res = run_bass_kernel_spmd(
        nc, in_maps, core_ids=list(range(N_CORES)), trace=trace
    )
    out = np.zeros((B, S, D), np.float32)
    for core in range(N_CORES):
        out[core // 4] += res.results[core]["out"].astype(np.float32)
    out += bo.astype(np.float32)
    return out, res


def kernel(x, Wq, Wk, Wv, Wo, bo):
    x, Wq, Wk, Wv, Wo, bo = (np.asarray(a) for a in (x, Wq, Wk, Wv, Wo, bo))
    out, _ = _run(x, Wq, Wk, Wv, Wo, bo, trace=False)
    return out


def kernel_traced(x, Wq, Wk, Wv, Wo, bo):
    """Same as kernel() but returns (out, BassKernelResults) with profiling."""
    x, Wq, Wk, Wv, Wo, bo = (np.asarray(a) for a in (x, Wq, Wk, Wv, Wo, bo))
    return _run(x, Wq, Wk, Wv, Wo, bo, trace=True)
